# revision 21
# baseline (speedup 1.0000x reference)
"""Trainium2 Bass kernel for nn_BagInput (segment_reduce + linear/relu + BatchNorm).

Computation (matches the reference):
    h   = relu(x @ W.T + b)                      [N_items, 128]
    agg = segment_mean(h, seg_ids, NB)           [NB, 128]   (empty bags -> 0)
    out = (agg - mean) * rsqrt(var + eps) * gamma + beta   (batch stats over bags)

Strategy (8 NeuronCores, data-parallel over items, bag-aligned shards):
  - Host: shard items at bag boundaries; per core, pack items into groups of
    T0=16 128-item tiles, padding each group so that group boundaries fall on
    bag boundaries (~0.5% padding). One group == one "window" of <=128 bags.
  - Host: pre-transpose x to feature-major (xT) so the per-tile [feat, item]
    blocks DMA directly as matmul stationary operands.
  - Device per 128-item tile:
      h_psum = xT0_t.T @ WT0 + xT1_t.T @ WT1      (PE, K=256 in 2 chunks)
      h_sbuf = relu(h_psum)                        (DVE / ACT alternating)
      S      = (iota_row == slot_ids_t)            (0/1 selector, GPSIMD/DVE)
      wps_w += S.T @ h_sbuf                        (PE; window accumulate in PSUM)
  - Window drain: agg = wps * (1/cnt)  (per-partition scalar); accumulate BN
    partial stats via mask.T @ [agg, agg^2] matmuls into PSUM.
  - AllReduce (8 cores) of the [1,256] stats; compute A = gamma*rsqrt(var+eps),
    B = beta - mean*A; broadcast via ones-matmul; out = agg*A + B; DMA out.
  - Host: gather per-(window,slot) rows back to global bag order.
"""

import numpy as np

N_CORES = 8
TILE = 128
FEAT = 256
BAG = 128
EPS = 1e-5

_NC_CACHE = {}
LAST_RESULTS = None  # BassKernelResults of the most recent run (for profiling)


# ----------------------------------------------------------------------------
# Host-side planning
# ----------------------------------------------------------------------------

def _plan_cores(seg_ids, n_bags, t0):
    """Split items/bags across cores at bag boundaries; pack groups of t0
    tiles per core such that each group covers whole bags (<=128 bags)."""
    gi = t0 * TILE
    n = seg_ids.shape[0]
    cuts = [0]
    bag_cuts = [0]
    for c in range(1, N_CORES):
        tgt = (n * c) // N_CORES
        bb = int(seg_ids[tgt])
        cut = int(np.searchsorted(seg_ids, bb, side="left"))
        cuts.append(cut)
        bag_cuts.append(bb)
    cuts.append(n)
    bag_cuts.append(n_bags)
    for c in range(N_CORES):
        if cuts[c + 1] <= cuts[c]:
            raise ValueError("degenerate core split")

    cores = []
    for c in range(N_CORES):
        i0, i1 = cuts[c], cuts[c + 1]
        b0, b1 = bag_cuts[c], bag_cuts[c + 1]
        seg = seg_ids[i0:i1]
        nloc = i1 - i0
        groups = []
        p = 0
        fb = b0
        while p < nloc:
            if p + gi >= nloc:
                e = nloc
                lbx = b1
            else:
                e = int(np.searchsorted(seg, seg[p + gi], side="left"))
                if e <= p:
                    raise ValueError("single bag larger than group size")
                lbx = int(seg[e - 1]) + 1
            if lbx - fb > TILE:
                raise ValueError(f"window spans {lbx - fb} bags > {TILE}")
            groups.append((p, e, fb, lbx))
            fb = lbx
            p = e
        cores.append(dict(i0=i0, i1=i1, b0=b0, b1=b1, groups=groups))
    return cores


def _host_prep(x, W, b, gamma, beta, seg_ids, bags_len):
    n_bags = bags_len.shape[0]
    plan = None
    for t0 in (16, 8, 4, 2):
        try:
            plan = _plan_cores(seg_ids, n_bags, t0)
            break
        except ValueError:
            continue
    if plan is None:
        raise ValueError("could not plan groups")
    gi = t0 * TILE

    ng = max(len(c["groups"]) for c in plan)
    nt = ng * t0
    npad = ng * gi

    cnt = np.maximum(bags_len, 1).astype(np.float32)
    recip_all = 1.0 / cnt

    in_maps = []
    for c in range(N_CORES):
        info = plan[c]
        i0 = info["i0"]
        seg = seg_ids[i0:info["i1"]]
        groups = info["groups"]

        idx = np.full(npad, -1, dtype=np.int64)
        slots = np.full(npad, 255.0, dtype=np.float32)
        recip = np.ones((ng, TILE), dtype=np.float32)
        mask = np.zeros((ng, TILE), dtype=np.float32)
        for g, (p, e, fb, lbx) in enumerate(groups):
            m = e - p
            idx[g * gi: g * gi + m] = i0 + p + np.arange(m)
            slots[g * gi: g * gi + m] = (seg[p:e] - fb).astype(np.float32)
            ns = lbx - fb
            recip[g, :ns] = recip_all[fb:lbx]
            mask[g, :ns] = 1.0

        xp = np.zeros((npad, FEAT), dtype=np.float32)
        valid = idx >= 0
        xp[valid] = x[idx[valid]]
        xT = np.ascontiguousarray(xp.T)
        del xp

        import os
        if int(os.environ.get("KERNEL_BF16H", "2")) >= 1:
            import ml_dtypes
            bf = ml_dtypes.bfloat16
            xTh = xT.astype(bf)
            xTl = (xT - xTh.astype(np.float32)).astype(bf)
            WTf = np.ascontiguousarray(W.T)
            WTh = WTf.astype(bf)
            WTl = (WTf - WTh.astype(np.float32)).astype(bf)
            in_maps.append({
                "xTh": np.ascontiguousarray(xTh),
                "xTl": np.ascontiguousarray(xTl),
                "WTh": np.ascontiguousarray(WTh),
                "WTl": np.ascontiguousarray(WTl),
                "slots": np.ascontiguousarray(slots.reshape(nt, TILE).T),
                "recip": np.ascontiguousarray(recip.T),
                "mask": np.ascontiguousarray(mask.T),
                "iota": np.ascontiguousarray(
                    np.tile(np.arange(TILE, dtype=np.float32), (TILE, 1))),
                "gamma_row": np.ascontiguousarray(gamma.reshape(1, BAG)),
                "beta_row": np.ascontiguousarray(beta.reshape(1, BAG)),
                "bias_bc": np.ascontiguousarray(
                    np.tile(b.reshape(1, BAG), (TILE, 1))),
            })
            continue

        in_maps.append({
            "xT": xT,
            "slots": np.ascontiguousarray(slots.reshape(nt, TILE).T),
            "recip": np.ascontiguousarray(recip.T),
            "mask": np.ascontiguousarray(mask.T),
            "WT": np.ascontiguousarray(W.T),
            "iota": np.ascontiguousarray(
                np.tile(np.arange(TILE, dtype=np.float32), (TILE, 1))),
            "gamma_row": np.ascontiguousarray(gamma.reshape(1, BAG)),
            "beta_row": np.ascontiguousarray(beta.reshape(1, BAG)),
            "bias_bc": np.ascontiguousarray(
                np.tile(b.reshape(1, BAG), (TILE, 1))),
        })
    return plan, t0, ng, in_maps, n_bags


# ----------------------------------------------------------------------------
# Device kernel
# ----------------------------------------------------------------------------

def _build_nc(ng, t0, n_bags, has_bias, relu_dve_mod=2, sbuild_dve_mod=4,
              use_f32r=False, use_bf16h=False, use_bf16seg=False):
    import concourse.bacc as bacc
    import concourse.tile as tile
    import concourse.mybir as mybir

    fp32 = mybir.dt.float32
    mmdt = mybir.dt.float32r if use_f32r else fp32
    bf16 = mybir.dt.bfloat16
    u8 = mybir.dt.uint8
    AOT = mybir.AluOpType
    AFT = mybir.ActivationFunctionType

    gi = t0 * TILE
    nt = ng * t0
    npad = ng * gi

    nc = bacc.Bacc("TRN2", target_bir_lowering=False, debug=False,
                   enable_asserts=False, num_devices=N_CORES)
    if use_bf16h:
        xTh = nc.dram_tensor("xTh", [FEAT, npad], bf16, kind="ExternalInput")
        xTl = nc.dram_tensor("xTl", [FEAT, npad], bf16, kind="ExternalInput")
        WTh_in = nc.dram_tensor("WTh", [FEAT, BAG], bf16, kind="ExternalInput")
        WTl_in = nc.dram_tensor("WTl", [FEAT, BAG], bf16, kind="ExternalInput")
    else:
        xT = nc.dram_tensor("xT", [FEAT, npad], mmdt, kind="ExternalInput")
        WT = nc.dram_tensor("WT", [FEAT, BAG], mmdt, kind="ExternalInput")
    slots = nc.dram_tensor("slots", [TILE, nt], fp32, kind="ExternalInput")
    recip = nc.dram_tensor("recip", [TILE, ng], fp32, kind="ExternalInput")
    mask = nc.dram_tensor("mask", [TILE, ng], fp32, kind="ExternalInput")
    iota_in = nc.dram_tensor("iota", [TILE, TILE], fp32, kind="ExternalInput")
    grow_in = nc.dram_tensor("gamma_row", [1, BAG], fp32, kind="ExternalInput")
    brow_in = nc.dram_tensor("beta_row", [1, BAG], fp32, kind="ExternalInput")
    bb_in = nc.dram_tensor("bias_bc", [TILE, BAG], fp32, kind="ExternalInput")
    out = nc.dram_tensor("out", [ng * TILE, BAG], fp32, kind="ExternalOutput")

    with tile.TileContext(nc) as tc:
        with tc.tile_pool(name="const", bufs=1) as constp, \
             tc.tile_pool(name="xa", bufs=4) as xa_p, \
             tc.tile_pool(name="xb", bufs=4) as xb_p, \
             tc.tile_pool(name="hsb", bufs=4) as hsb_p, \
             tc.tile_pool(name="Sp", bufs=4) as s_p, \
             tc.tile_pool(name="agg", bufs=1) as agg_p, \
             tc.tile_pool(name="agg2", bufs=2) as agg2_p, \
             tc.tile_pool(name="outp", bufs=2) as out_p, \
             tc.tile_pool(name="small", bufs=1) as small_p, \
             tc.tile_pool(name="hps", bufs=3, space="PSUM") as hps_p, \
             tc.tile_pool(name="wpsp", bufs=2, space="PSUM") as wps_p, \
             tc.tile_pool(name="spsa", bufs=1, space="PSUM") as sps_a_p, \
             tc.tile_pool(name="spsb", bufs=1, space="PSUM") as sps_b_p, \
             tc.tile_pool(name="abps", bufs=1, space="PSUM") as ab_p, \
             tc.tile_pool(name="dram", bufs=1, space="DRAM") as dram_p:

            if use_bf16h:
                wt0h = constp.tile([128, BAG], bf16)
                nc.sync.dma_start(wt0h[:], WTh_in[0:128, :])
                wt1h = constp.tile([128, BAG], bf16)
                nc.sync.dma_start(wt1h[:], WTh_in[128:256, :])
                wt0l = constp.tile([128, BAG], bf16)
                nc.sync.dma_start(wt0l[:], WTl_in[0:128, :])
                wt1l = constp.tile([128, BAG], bf16)
                nc.sync.dma_start(wt1l[:], WTl_in[128:256, :])
            else:
                wt0 = constp.tile([128, BAG], mmdt)
                nc.sync.dma_start(wt0[:], WT[0:128, :])
                wt1 = constp.tile([128, BAG], mmdt)
                nc.sync.dma_start(wt1[:], WT[128:256, :])
            iota_sb = constp.tile([TILE, TILE], fp32)
            nc.sync.dma_start(iota_sb[:], iota_in[:, :])
            recip_sb = constp.tile([TILE, ng], fp32)
            nc.sync.dma_start(recip_sb[:], recip[:, :])
            mask_sb = constp.tile([TILE, ng], fp32)
            nc.sync.dma_start(mask_sb[:], mask[:, :])
            slots_sb = constp.tile([TILE, nt], fp32)
            nc.sync.dma_start(slots_sb[:], slots[:, :])
            segdt = bf16 if use_bf16seg else mmdt
            zeros_f32 = constp.tile([TILE, TILE], fp32)
            nc.vector.memset(zeros_f32[:], 0.0)
            if use_f32r or use_bf16seg:
                zeros_S = constp.tile([TILE, TILE], segdt)
                nc.vector.tensor_copy(zeros_S[:], zeros_f32[:])
            else:
                zeros_S = zeros_f32
            ones_row = constp.tile([1, TILE], fp32)
            nc.vector.memset(ones_row[:], 1.0)
            grow = constp.tile([1, BAG], fp32)
            nc.sync.dma_start(grow[:], grow_in[:, :])
            brow = constp.tile([1, BAG], fp32)
            nc.sync.dma_start(brow[:], brow_in[:, :])
            if has_bias:
                bias_bc = constp.tile([TILE, BAG], fp32)
                nc.sync.dma_start(bias_bc[:], bb_in[:, :])

            stats_a = sps_a_p.tile([1, BAG], fp32)
            stats_b = sps_b_p.tile([1, BAG], fp32)
            agg_big = agg_p.tile([TILE, ng * BAG], fp32)


            # ---------------- phase 1: streamed quads (4 tiles each) -------
            # Process 4 item-tiles at a time: one [128, 512] h PSUM bank, one
            # wide relu (DVE/ACT alternating), one wide selector build
            # (tensor_tensor vs a stride-0 broadcast of 4 slot columns).
            QT = 4                  # tiles per quad
            assert t0 % QT == 0
            qpw = t0 // QT          # quads per window
            nq = nt // QT
            WID = QT * BAG          # 512

            import concourse.bass as bass_mod
            wps_tiles = {}
            xa = xb = None
            prev = None
            for q in range(nq + 1):
                if q < nq:
                    w, jq = divmod(q, qpw)
                    if jq == 0:
                        if use_bf16h:
                            xa = xa_p.tile([128, 2 * gi], bf16, tag="xa")
                            nc.sync.dma_start(
                                xa[:, 0:gi], xTh[0:128, w * gi:(w + 1) * gi])
                            nc.sync.dma_start(
                                xa[:, gi:2 * gi], xTl[0:128, w * gi:(w + 1) * gi])
                            xb = xb_p.tile([128, 2 * gi], bf16, tag="xb")
                            nc.sync.dma_start(
                                xb[:, 0:gi], xTh[128:256, w * gi:(w + 1) * gi])
                            nc.sync.dma_start(
                                xb[:, gi:2 * gi], xTl[128:256, w * gi:(w + 1) * gi])
                        else:
                            xa = xa_p.tile([128, gi], mmdt)
                            nc.sync.dma_start(xa[:], xT[0:128, w * gi:(w + 1) * gi])
                            xb = xb_p.tile([128, gi], mmdt)
                            nc.sync.dma_start(xb[:], xT[128:256, w * gi:(w + 1) * gi])
                        wt_ps = wps_p.tile([TILE, BAG], fp32)
                        wps_tiles[w] = wt_ps
                        nc.tensor.matmul(wt_ps[:], zeros_S[:], zeros_S[:, 0:BAG],
                                         start=True, stop=False)
                    hps = hps_p.tile([TILE, WID], fp32)
                    for j in range(QT):
                        c0 = (jq * QT + j) * 128
                        o = (j * BAG, (j + 1) * BAG)
                        if use_bf16h:
                            nc.tensor.matmul(hps[:, o[0]:o[1]],
                                             xa[:, c0:c0 + 128], wt0h[:],
                                             start=True, stop=False)
                            nc.tensor.matmul(hps[:, o[0]:o[1]],
                                             xa[:, c0:c0 + 128], wt0l[:],
                                             start=False, stop=False)
                            nc.tensor.matmul(hps[:, o[0]:o[1]],
                                             xb[:, c0:c0 + 128], wt1h[:],
                                             start=False, stop=False)
                            nc.tensor.matmul(hps[:, o[0]:o[1]],
                                             xb[:, c0:c0 + 128], wt1l[:],
                                             start=False, stop=False)
                            nc.tensor.matmul(hps[:, o[0]:o[1]],
                                             xa[:, gi + c0:gi + c0 + 128], wt0h[:],
                                             start=False, stop=False)
                            nc.tensor.matmul(hps[:, o[0]:o[1]],
                                             xb[:, gi + c0:gi + c0 + 128], wt1h[:],
                                             start=False, stop=True)
                        else:
                            nc.tensor.matmul(hps[:, o[0]:o[1]],
                                             xa[:, c0:c0 + 128], wt0[:],
                                             start=True, stop=False)
                            nc.tensor.matmul(hps[:, o[0]:o[1]],
                                             xb[:, c0:c0 + 128], wt1[:],
                                             start=False, stop=True)
                    if use_bf16seg:
                        hsb = hsb_p.tile([TILE, WID], bf16, tag="hsb_hi")
                        hlo = hsb_p.tile([TILE, WID], bf16, tag="hsb_lo")
                        nc.scalar.activation(hsb[:], hps[:], AFT.Relu)
                        nc.vector.scalar_tensor_tensor(
                            hlo[:], hps[:], 0.0, hsb[:], AOT.max, AOT.subtract)
                    else:
                        hlo = None
                        hsb = hsb_p.tile([TILE, WID], mmdt)
                    if use_bf16seg:
                        pass
                    elif has_bias:
                        bias4 = bass_mod.AP(
                            tensor=bias_bc.tensor, offset=bias_bc.offset,
                            ap=[bias_bc.ap[0], [0, QT], bias_bc.ap[1]])
                        nc.vector.tensor_tensor(
                            hsb[:].rearrange("p (a b) -> p a b", a=QT),
                            hps[:].rearrange("p (a b) -> p a b", a=QT),
                            bias4, AOT.add)
                        nc.vector.tensor_scalar_max(hsb[:], hsb[:], 0.0)
                    else:
                        if q % 2 == 0:
                            nc.vector.tensor_scalar_max(hsb[:], hps[:], 0.0)
                        else:
                            nc.scalar.activation(hsb[:], hps[:], AFT.Relu)
                    # wide selector: S[p, a*128 + f] = (iota[f] == slots[p, t0q+a])
                    S = s_p.tile([TILE, WID], segdt)
                    scol = slots_sb[:, q * QT:(q + 1) * QT]
                    srep = bass_mod.AP(tensor=scol.tensor, offset=scol.offset,
                                       ap=[scol.ap[0], scol.ap[1], [0, BAG]])
                    ibase = iota_sb[:]
                    irep = bass_mod.AP(tensor=ibase.tensor, offset=ibase.offset,
                                       ap=[ibase.ap[0], [0, QT], ibase.ap[1]])
                    nc.vector.tensor_tensor(
                        S[:].rearrange("p (a b) -> p a b", a=QT),
                        irep, srep, AOT.is_equal)
                    cur = (q, S, hsb, hlo, w, jq == qpw - 1)
                else:
                    cur = None
                if prev is not None:
                    pq, pS, phsb, phlo, pw, plast = prev
                    for j in range(QT):
                        last = plast and j == QT - 1
                        nc.tensor.matmul(wps_tiles[pw][:],
                                         pS[:, j * BAG:(j + 1) * BAG],
                                         phsb[:, j * BAG:(j + 1) * BAG],
                                         start=False,
                                         stop=(last and phlo is None))
                        if phlo is not None:
                            nc.tensor.matmul(wps_tiles[pw][:],
                                             pS[:, j * BAG:(j + 1) * BAG],
                                             phlo[:, j * BAG:(j + 1) * BAG],
                                             start=False, stop=last)
                    if plast:
                        aggw = agg_big[:, pw * BAG:(pw + 1) * BAG]
                        nc.scalar.activation(aggw, wps_tiles[pw][:], AFT.Copy,
                                             scale=recip_sb[:, pw:pw + 1])
                        a2 = agg2_p.tile([TILE, BAG], fp32)
                        nc.scalar.square(a2[:], aggw)
                        nc.tensor.matmul(stats_a[:], mask_sb[:, pw:pw + 1], aggw,
                                         start=(pw == 0), stop=(pw == ng - 1))
                        nc.tensor.matmul(stats_b[:], mask_sb[:, pw:pw + 1], a2[:],
                                         start=(pw == 0), stop=(pw == ng - 1))
                        del wps_tiles[pw]
                prev = cur

            # ---------------- stats all-reduce + params ----------------
            stats_sb = small_p.tile([1, 2 * BAG], fp32)
            nc.vector.tensor_copy(stats_sb[0:1, 0:BAG], stats_a[:])
            nc.vector.tensor_copy(stats_sb[0:1, BAG:2 * BAG], stats_b[:])
            cc_in = dram_p.tile([1, 2 * BAG], fp32)
            cc_out = dram_p.tile([1, 2 * BAG], fp32)
            nc.sync.dma_start(cc_in[:], stats_sb[:])
            nc.gpsimd.collective_compute(
                "AllReduce", AOT.add,
                replica_groups=[list(range(N_CORES))],
                ins=[cc_in.opt()], outs=[cc_out.opt()])
            gstats = small_p.tile([1, 2 * BAG], fp32)
            nc.sync.dma_start(gstats[:], cc_out[:])

            inv_nb = 1.0 / float(n_bags)
            mean = small_p.tile([1, BAG], fp32)
            nc.vector.tensor_scalar_mul(mean[:], gstats[0:1, 0:BAG], inv_nb)
            ex2 = small_p.tile([1, BAG], fp32)
            nc.vector.tensor_scalar_mul(ex2[:], gstats[0:1, BAG:2 * BAG], inv_nb)
            m2 = small_p.tile([1, BAG], fp32)
            nc.vector.tensor_tensor(m2[:], mean[:], mean[:], AOT.mult)
            vareps = small_p.tile([1, BAG], fp32)
            nc.vector.tensor_tensor(vareps[:], ex2[:], m2[:], AOT.subtract)
            nc.vector.tensor_scalar_add(vareps[:], vareps[:], EPS)
            rec = small_p.tile([1, BAG], fp32)
            nc.vector.reciprocal(rec[:], vareps[:])
            inv = small_p.tile([1, BAG], fp32)
            nc.scalar.sqrt(inv[:], rec[:])
            ab_row = small_p.tile([1, 2 * BAG], fp32)
            nc.vector.tensor_tensor(ab_row[0:1, 0:BAG], inv[:], grow[:], AOT.mult)
            mA = small_p.tile([1, BAG], fp32)
            nc.vector.tensor_tensor(mA[:], mean[:], ab_row[0:1, 0:BAG], AOT.mult)
            nc.vector.tensor_tensor(ab_row[0:1, BAG:2 * BAG], brow[:], mA[:],
                                    AOT.subtract)
            ab_ps = ab_p.tile([TILE, 2 * BAG], fp32)
            nc.tensor.matmul(ab_ps[:], ones_row[:], ab_row[:], start=True, stop=True)
            ab_sb = constp.tile([TILE, 2 * BAG], fp32)
            nc.vector.tensor_copy(ab_sb[:], ab_ps[:])

            # ---------------- phase 2: normalize + store ----------------
            a_col = ab_sb[:, 0:BAG]
            b_col = ab_sb[:, BAG:2 * BAG]
            w2 = 0
            while w2 < ng:
                nw = min(4, ng - w2)
                wid2 = nw * BAG
                arep = bass_mod.AP(tensor=a_col.tensor, offset=a_col.offset,
                                   ap=[a_col.ap[0], [0, nw], a_col.ap[1]])
                brep = bass_mod.AP(tensor=b_col.tensor, offset=b_col.offset,
                                   ap=[b_col.ap[0], [0, nw], b_col.ap[1]])
                ot = out_p.tile([TILE, 4 * BAG], fp32)
                src = agg_big[:, w2 * BAG:(w2 + nw) * BAG]
                nc.vector.tensor_tensor(
                    ot[:, 0:wid2].rearrange("p (a b) -> p a b", a=nw),
                    src.rearrange("p (a b) -> p a b", a=nw), arep, AOT.mult)
                nc.vector.tensor_tensor(
                    ot[:, 0:wid2].rearrange("p (a b) -> p a b", a=nw),
                    ot[:, 0:wid2].rearrange("p (a b) -> p a b", a=nw),
                    brep, AOT.add)
                # out rows for nw windows are contiguous: [w2*128, (w2+nw)*128)
                nc.sync.dma_start(
                    out[w2 * TILE:(w2 + nw) * TILE, :].rearrange(
                        "(a p) b -> p a b", p=TILE),
                    ot[:, 0:wid2].rearrange("p (a b) -> p a b", a=nw))
                w2 += nw

    nc.compile()
    return nc


# ----------------------------------------------------------------------------
# Entry point
# ----------------------------------------------------------------------------

def kernel(**inputs):
    global LAST_RESULTS
    from concourse.bass_utils import run_bass_kernel_spmd

    x = np.asarray(inputs["x"], dtype=np.float32)
    W = np.asarray(inputs["W"], dtype=np.float32)
    b = np.asarray(inputs["b"], dtype=np.float32)
    gamma = np.asarray(inputs["gamma"], dtype=np.float32)
    beta = np.asarray(inputs["beta"], dtype=np.float32)
    seg_ids = np.asarray(inputs["seg_ids"]).astype(np.int64)
    bags_len = np.asarray(inputs["bags_len"]).astype(np.int64)

    plan, t0, ng, in_maps, n_bags = _host_prep(
        x, W, b, gamma, beta, seg_ids, bags_len)
    has_bias = bool(np.any(b != 0))

    import os
    use_f32r = os.environ.get("KERNEL_F32R", "0") == "1"
    bf_mode = int(os.environ.get("KERNEL_BF16H", "2"))
    use_bf16h = bf_mode >= 1
    use_bf16seg = bf_mode >= 2 and not has_bias
    key = (ng, t0, n_bags, has_bias, use_f32r, use_bf16h, use_bf16seg)
    if key not in _NC_CACHE:
        _NC_CACHE[key] = _build_nc(ng, t0, n_bags, has_bias,
                                   use_f32r=use_f32r, use_bf16h=use_bf16h,
                                   use_bf16seg=use_bf16seg)
    nc = _NC_CACHE[key]

    res = run_bass_kernel_spmd(nc, in_maps, core_ids=list(range(N_CORES)))
    LAST_RESULTS = res

    out_full = np.zeros((n_bags, BAG), dtype=np.float32)
    for c in range(N_CORES):
        oc = res.results[c]["out"]
        for g, (p, e, fb, lbx) in enumerate(plan[c]["groups"]):
            ns = lbx - fb
            out_full[fb:lbx] = oc[g * TILE: g * TILE + ns]
    return out_full
